# revision 8
# baseline (speedup 1.0000x reference)
"""DisturbLabel cross-entropy (mean NLL with stochastically disturbed labels)
on 8 Trainium2 NeuronCores.

Math:  mean_b [ logsumexp(output[b, :]) - output[b, new_target[b]] ]
where new_target is the reference's deterministic jax.random.key(42) disturb
draw.  The heavy part (logsumexp over an 8192x32000 f32 matrix, ~1 GiB HBM
read) runs on the NeuronCores, data-parallel over the batch dim (1024 rows
per core).  The O(B) parts (label sampling, target-logit gather, log, mean)
run on host.

Device kernel per core: stream [128, 4000] f32 chunks, scalar-engine Exp
with fused accum_out row-sum, accumulate 8 chunk sums per row-tile with one
small vector reduce, store per-row sum-of-exp ([128, 8] f32, 4 KiB).
"""

from contextlib import ExitStack

import numpy as np

B = 8192
C = 32000
N_CORES = 8
ROWS_PER_CORE = B // N_CORES  # 1024
P = 128                       # SBUF partitions (rows per tile)
N_RT = ROWS_PER_CORE // P     # 8 row-tiles per core
W = 4000                      # chunk width (cols); 128*4000*4B = 2 MiB per DMA
N_CHUNK = C // W              # 8 chunks per row-tile
NOISY_RATE = 0.1

# test.py can flip these before calling kernel() to get a profile
TRACE = False
LAST_RESULTS = None

_nc_cache = None


NBUF = 8  # SBUF chunk slots (double-buffer depth); one semaphore per slot


def _build_bass():
    """Raw-bass pipeline.  This walrus (neuronxcc coreV2 codegen) permits at
    most ONE sync wait per instruction, which rules out Tile's scheduler
    (its slot-WAR + lane-FIFO waits routinely pair up).  Structure:

      SP engine:  64 load DMAs (one [128, W] f32 chunk each, HWDGE FIFO);
                  load n>=NBUF first waits s_free >= n-NBUF+1 (slot WAR).
      ACT engine: per chunk: wait slot sem >= 16*(uses), then in-place Exp
                  with accum_out -> per-row chunk sum; inc s_free.
                  Per row-tile: out-DMA of the [128, N_CHUNK] sums on the
                  ACT HWDGE ring.  Final wait for out-DMA completion.

    Per-slot DMA semaphores (not one shared sem) because a shared counter
    gets partial credit from later DMAs' per-SDMA-engine increments; with
    one outstanding DMA per slot the wait value is unambiguous.
    """
    global _nc_cache
    if _nc_cache is not None:
        return _nc_cache

    import concourse.bass as bass
    from concourse import mybir

    f32 = mybir.dt.float32
    NTOT = N_RT * N_CHUNK

    nc = bass.Bass("TRN2", debug=False, num_devices=N_CORES)
    x = nc.dram_tensor("x", [ROWS_PER_CORE, C], f32, kind="ExternalInput").ap()
    out = nc.dram_tensor(
        "out", [N_RT, P, N_CHUNK], f32, kind="ExternalOutput"
    ).ap()
    xbuf = nc.alloc_sbuf_tensor("xbuf", [P, NBUF, W], f32).ap()
    accs = nc.alloc_sbuf_tensor("accs", [P, NTOT], f32).ap()
    warm = nc.alloc_sbuf_tensor("warm", [P, 1], f32).ap()

    with ExitStack() as ctx:
        block = ctx.enter_context(nc.Block())
        s_slot = [
            ctx.enter_context(nc.semaphore(f"s_slot{i}")) for i in range(NBUF)
        ]
        s_free = ctx.enter_context(nc.semaphore("s_free"))
        s_out = ctx.enter_context(nc.semaphore("s_out"))

        @block.sync
        def _(sp):
            for n in range(NTOT):
                rt, ci = divmod(n, N_CHUNK)
                slot = n % NBUF
                if n >= NBUF:
                    sp.wait_ge(s_free, n - NBUF + 1)
                sp.dma_start(
                    out=xbuf[:, slot],
                    in_=x[rt * P : (rt + 1) * P, ci * W : (ci + 1) * W],
                ).then_inc(s_slot[slot], 16)

        @block.scalar
        def _(act):
            # dependency-free warmup: the ACT_TABLE_LOAD for Exp that walrus
            # inserts before the first ACTIVATE lands on an instruction with
            # a free sync-wait slot (consts are barrier-synced at init)
            act.activation(
                out=warm,
                in_=nc.const_aps.tensor(0.0, [P, 1]),
                func=mybir.ActivationFunctionType.Exp,
            )
            for n in range(NTOT):
                rt, ci = divmod(n, N_CHUNK)
                slot = n % NBUF
                act.wait_ge(s_slot[slot], 16 * (n // NBUF + 1))
                act.activation(
                    out=xbuf[:, slot],
                    in_=xbuf[:, slot],
                    func=mybir.ActivationFunctionType.Exp,
                    accum_out=accs[:, n : n + 1],
                ).then_inc(s_free, 1)
                if ci == N_CHUNK - 1:
                    # the exps' retire-time s_free incs guarantee the accum
                    # writes landed before the DMA engine reads accs
                    act.wait_ge(s_free, (rt + 1) * N_CHUNK)
                    act.dma_start(
                        out=out[rt],
                        in_=accs[:, rt * N_CHUNK : (rt + 1) * N_CHUNK],
                    ).then_inc(s_out, 16)
            act.wait_ge(s_out, 16 * N_RT)

    _nc_cache = nc
    return nc


def _disturbed_targets(target_i32: np.ndarray) -> np.ndarray:
    """Replicate reference.py's label disturbance bit-exactly (jax threefry
    is platform-deterministic)."""
    import jax
    import jax.numpy as jnp

    bound = (C - 1.0) / float(C) * NOISY_RATE
    cpu = jax.devices("cpu")[0]
    with jax.default_device(cpu):
        key = jax.random.key(42)
        kr, kd = jax.random.split(key)
        r = jax.random.uniform(kr, (B,), dtype=jnp.float32)
        d = jax.random.randint(kd, (B,), 0, C - 1)
        tgt = jnp.asarray(target_i32)
        dlabel = d + (d >= tgt).astype(jnp.int32)
        new_target = jnp.where(r < bound, dlabel, tgt)
        return np.asarray(new_target)


def kernel(output: np.ndarray, target: np.ndarray) -> np.ndarray:
    global LAST_RESULTS
    from concourse import bass_utils

    output = np.asarray(output)
    assert output.shape == (B, C) and output.dtype == np.float32
    tgt_i32 = np.asarray(target).astype(np.int32)

    new_target = _disturbed_targets(tgt_i32)
    picked = output[np.arange(B), new_target].astype(np.float64)

    nc = _build_bass()
    in_maps = [
        {"x": np.ascontiguousarray(output[k * ROWS_PER_CORE : (k + 1) * ROWS_PER_CORE])}
        for k in range(N_CORES)
    ]
    res = bass_utils.run_bass_kernel_spmd(
        nc, in_maps, list(range(N_CORES)), trace=TRACE
    )
    LAST_RESULTS = res

    outs = np.stack([r["out"] for r in res.results])  # [N_CORES, N_RT, P, N_CHUNK]
    # global row = k*1024 + rt*128 + p
    sumexp = outs.astype(np.float64).sum(axis=-1).reshape(B)
    logz = np.log(sumexp)
    val = logz.mean() - picked.mean()
    return np.asarray(val, dtype=np.float32)
